# revision 11
# baseline (speedup 1.0000x reference)
"""Trainium2 Bass kernel for 4-directional cumulative-max corner pooling.

reference: p = x[:, :16]; out = concat([x, cummax_H(p), cummax_H_rev(p),
                                        cummax_W(p), cummax_W_rev(p)], axis=1)
x: [32, 64, 128, 128] f32 -> out: [32, 128, 128, 128] f32

Strategy (v3): the whole kernel is DVE-bound — the only engine that can
run a prefix recurrence.  The stock TensorTensorScanArith runs at ~2.15
ns/column; a custom DVE uOp (SEGMAX_SCAN, below) runs the same
segmented cummax at 1 elem/cycle (~1.11 ns/col) by hand-patching the
subdim step state to reset the scan latch (max(-FLT_MAX, x)) at each
page boundary of a [P, 16, 128] access pattern.  Bit-exact vs
np.maximum.accumulate on the bf16 data.

Per core: 4 batches x 4 directions x 2048 columns x 1.11 ns = ~36 us of
DVE, with ~12 MB of DMA (~34 us at the 358 GB/s roofline) hidden under
it.  No PE/PSUM: the host supplies both layouts of the picked channels
([h, c w] and [w, c h], bf16), and 'left'/'up' run the scan through
reversed access patterns (measured free on TRN2) writing through
equally reversed output APs, so every output region lands in natural
layout.  The out[:, :64] = x passthrough never touches the device.

Sharding: data-parallel over batch, 4 batches per core on 8 cores; no
cross-core communication.
"""

import numpy as np
from contextlib import ExitStack

import ml_dtypes

import concourse.bass as bass
import concourse.bacc as bacc
import concourse.mybir as mybir
from concourse.tile import TileContext
from concourse.bass_utils import run_bass_kernel_spmd

import concourse.dve_spec as _ds
import concourse.dve_ops as _do
from concourse.dve_spec import Spec as _Spec, Src0 as _Src0, scan as _scan
from concourse.dve_spec import AluOp as _AluOp

B_TOTAL, C_IN, H, W = 32, 64, 128, 128
PICK = 16
N_CORES = 8
B_PER = B_TOTAL // N_CORES
C_OUT = C_IN + 4 * PICK
F32 = mybir.dt.float32
BF16 = mybir.dt.bfloat16
NP_BF16 = ml_dtypes.bfloat16
CW = PICK * W  # 2048, free size of one direction's tile


# --- SEGMAX_SCAN: custom DVE op — segmented cummax along the free dim. ---
#
# Spec body is a plain max-scan; the subdim step state is hand-patched to
# reset the scan latch at each page boundary: on SUB_DIM_DONE the scan
# stage computes max(-FLT_MAX, x) = x for the first element of the new
# page, instead of max(state, x).  With in0/out as [P, S, N] APs
# (subdim=True, no AP coalescing), this is np.maximum.accumulate per
# page at 1 elem/cycle/partition.

def _segmax_ref(in0, in1, s0, s1, imm2):
    return np.maximum.accumulate(in0.astype(np.float32), axis=-1)


class _SegmaxDveOp(_do.DveOp):
    def compile(self, ver):
        key = (self.name, ver)
        if (r := _do._COMPILE_CACHE.get(key)) is not None:
            return r
        orig = _ds._scan_overrides

        def patched(scans, node_stage):
            seed, step = orig(scans, node_stage)
            for sc in scans:
                step[node_stage[sc]] = _ds._Stage(
                    _ds.AluOp.MAX, _ds.MaxNeg, _ds.Src0
                )
            return seed, step

        _ds._scan_overrides = patched
        try:
            uops = _ds.lower(self.spec, ver=ver)
        finally:
            _ds._scan_overrides = orig
        result = _do.DveOpSpec(
            name=self.name,
            opcode=_do.get_dve_sub_opcode(self.name),
            uops=uops,
            rd1_en=_do.has_src1(self.spec),
        )
        _do._COMPILE_CACHE[key] = result
        return result


def _get_segmax_op():
    for op in _do.OPS:
        if op.name == "SEGMAX_SCAN":
            return op
    op = _SegmaxDveOp(
        "SEGMAX_SCAN",
        _Spec(body=_scan(_AluOp.MAX, _Src0), reference=_segmax_ref),
        subdim=True,
        uops_sha={},
    )
    _do.OPS.append(op)
    _do._SUB_OPCODE_FOR_NAME[op.name] = _do._CUSTOM_DVE_ROW_BASE + _do.OPS.index(op)
    _do.CUSTOM_DVE_SPECS[op.name] = op.spec
    return op


def segmax_scan(nc, out3, in3):
    """out3/in3: [P, S, N] APs — cummax along N, reset per S page."""
    return nc.vector._custom_dve(_get_segmax_op(), out=out3, in0=in3)


# --- kernel body ---

def _emit(
    ctx: ExitStack,
    tc: TileContext,
    xh: bass.AP,
    xv: bass.AP,
    out_all: bass.AP,
    reps: int = 1,
) -> None:
    nc = tc.nc

    in_pool = ctx.enter_context(tc.tile_pool(name="tin", bufs=3))
    out_pool = ctx.enter_context(tc.tile_pool(name="tout", bufs=3))

    def seg3(ap):
        return ap.rearrange("p (s n) -> p s n", n=W)

    HF = CW // 2  # 1024: an 8-channel half

    for _rep in range(reps):
        for b in range(B_PER):
            first = _rep == 0 and b == 0
            last = _rep == reps - 1 and b == B_PER - 1

            th = in_pool.tile([128, CW], BF16, tag="th")
            tv = in_pool.tile([128, CW], BF16, tag="tv")
            if first:
                # Split the first load so the first scan (channels 0-7)
                # starts after 256 KB instead of 512 KB.  The two halves
                # go out on different trigger engines so their queue
                # entries enqueue back-to-back, and the tv copy (not
                # needed until the 3rd scan) trails both.
                nc.sync.dma_start(out=th[:, 0:HF], in_=xh[b, :, 0:HF])
                nc.scalar.dma_start(out=th[:, HF:CW], in_=xh[b, :, HF:CW])
                nc.sync.dma_start(out=tv[:], in_=xv[b])
            else:
                nc.sync.dma_start(out=th[:], in_=xh[b])
                nc.scalar.dma_start(out=tv[:], in_=xv[b])

            ob = out_pool.tile([128, 4 * CW], BF16, tag="ob")
            # right: forward segmented cummax over [h, (c w)].
            if first:
                segmax_scan(nc, seg3(ob[:, 0:HF]), seg3(th[:, 0:HF]))
                segmax_scan(nc, seg3(ob[:, HF:CW]), seg3(th[:, HF:CW]))
            else:
                segmax_scan(nc, seg3(ob[:, 0:CW]), seg3(th[:]))
            # left: the same scan through fully-reversed in/out APs; the
            # two reversals cancel, so the region lands in natural (c, w).
            segmax_scan(
                nc,
                seg3(ob[:, CW : 2 * CW])[:, ::-1, ::-1],
                seg3(th[:])[:, ::-1, ::-1],
            )
            # down/up: identical over the host-transposed [w, (c h)] copy.
            segmax_scan(nc, seg3(ob[:, 2 * CW : 3 * CW]), seg3(tv[:]))
            uo = seg3(ob[:, 3 * CW : 4 * CW])[:, ::-1, ::-1]
            ui = seg3(tv[:])[:, ::-1, ::-1]
            if last:
                # Split the final scans+stores so the tail drains 256 KB.
                nc.sync.dma_start(
                    out=out_all[b, :, 0 : 2 * CW], in_=ob[:, 0 : 2 * CW]
                )
                segmax_scan(nc, uo[:, 0:8], ui[:, 0:8])
                nc.scalar.dma_start(
                    out=out_all[b, :, 2 * CW : 3 * CW + HF],
                    in_=ob[:, 2 * CW : 3 * CW + HF],
                )
                segmax_scan(nc, uo[:, 8:16], ui[:, 8:16])
                nc.scalar.dma_start(
                    out=out_all[b, :, 3 * CW + HF : 4 * CW],
                    in_=ob[:, 3 * CW + HF : 4 * CW],
                )
            else:
                segmax_scan(nc, uo, ui)
                eng = nc.sync if b % 2 == 0 else nc.scalar
                eng.dma_start(out=out_all[b], in_=ob[:])


def build_nc(reps: int = 1) -> bass.Bass:
    # Bacc (not raw Bass): its compile() legalizes sync waits for TRN2.
    nc = bacc.Bacc("TRN2", target_bir_lowering=False, debug=False)
    # xh: picked channels as [b, h, (c w)]; xv: as [b, w, (c h)] (bf16).
    xh = nc.declare_dram_parameter("xh", [B_PER, H, CW], BF16, isOutput=False)
    xv = nc.declare_dram_parameter("xv", [B_PER, W, CW], BF16, isOutput=False)
    # out_all: [b, h|w, (right|left, c, w | down|up, c, h)].
    out_all = nc.declare_dram_parameter(
        "out_all", [B_PER, H, 4 * CW], BF16, isOutput=True
    )
    with TileContext(nc) as tc:
        with ExitStack() as ctx:
            _emit(ctx, tc, xh, xv, out_all, reps=reps)
    nc.compile()
    return nc


def make_in_maps(x: np.ndarray) -> list[dict[str, np.ndarray]]:
    p = x[:, :PICK]
    xh = np.ascontiguousarray(p.transpose(0, 2, 1, 3)).astype(NP_BF16)
    xv = np.ascontiguousarray(p.transpose(0, 3, 1, 2)).astype(NP_BF16)
    xh = xh.reshape(B_TOTAL, H, CW)
    xv = xv.reshape(B_TOTAL, W, CW)
    return [
        {
            "xh": xh[k * B_PER : (k + 1) * B_PER],
            "xv": xv[k * B_PER : (k + 1) * B_PER],
        }
        for k in range(N_CORES)
    ]


def kernel(x: np.ndarray, **_unused) -> np.ndarray:
    assert x.shape == (B_TOTAL, C_IN, H, W), x.shape
    nc = build_nc()
    res = run_bass_kernel_spmd(nc, make_in_maps(x), list(range(N_CORES)))

    out = np.empty((B_TOTAL, C_OUT, H, W), np.float32)
    out[:, :C_IN] = x
    oa = np.concatenate([r["out_all"] for r in res.results], axis=0)
    rl, du = oa[:, :, 0 : 2 * CW], oa[:, :, 2 * CW : 4 * CW]
    rl = rl.reshape(B_TOTAL, H, 2, PICK, W).astype(np.float32)
    du = du.reshape(B_TOTAL, W, 2, PICK, H).astype(np.float32)
    out[:, C_IN : C_IN + PICK] = du[:, :, 0].transpose(0, 2, 3, 1)  # down
    out[:, C_IN + PICK : C_IN + 2 * PICK] = du[:, :, 1].transpose(0, 2, 3, 1)  # up
    out[:, C_IN + 2 * PICK : C_IN + 3 * PICK] = rl[:, :, 0].transpose(0, 2, 1, 3)
    out[:, C_IN + 3 * PICK :] = rl[:, :, 1].transpose(0, 2, 1, 3)  # left
    return out


# revision 12
# speedup vs baseline: 1.0186x; 1.0186x over previous
"""Trainium2 Bass kernel for 4-directional cumulative-max corner pooling.

reference: p = x[:, :16]; out = concat([x, cummax_H(p), cummax_H_rev(p),
                                        cummax_W(p), cummax_W_rev(p)], axis=1)
x: [32, 64, 128, 128] f32 -> out: [32, 128, 128, 128] f32

Strategy (v3): the whole kernel is DVE-bound — the only engine that can
run a prefix recurrence.  The stock TensorTensorScanArith runs at ~2.15
ns/column; a custom DVE uOp (SEGMAX_SCAN, below) runs the same
segmented cummax at 1 elem/cycle (~1.11 ns/col) by hand-patching the
subdim step state to reset the scan latch (max(-FLT_MAX, x)) at each
page boundary of a [P, 16, 128] access pattern.  Bit-exact vs
np.maximum.accumulate on the bf16 data.

Per core: 4 batches x 4 directions x 2048 columns x 1.11 ns = ~36 us of
DVE, with ~12 MB of DMA (~34 us at the 358 GB/s roofline) hidden under
it.  No PE/PSUM: the host supplies both layouts of the picked channels
([h, c w] and [w, c h], bf16), and 'left'/'up' run the scan through
reversed access patterns (measured free on TRN2) writing through
equally reversed output APs, so every output region lands in natural
layout.  The out[:, :64] = x passthrough never touches the device.

Sharding: data-parallel over batch, 4 batches per core on 8 cores; no
cross-core communication.
"""

import numpy as np
from contextlib import ExitStack

import ml_dtypes

import concourse.bass as bass
import concourse.bacc as bacc
import concourse.mybir as mybir
from concourse.tile import TileContext
from concourse.bass_utils import run_bass_kernel_spmd

import concourse.dve_spec as _ds
import concourse.dve_ops as _do
from concourse.dve_spec import Spec as _Spec, Src0 as _Src0, scan as _scan
from concourse.dve_spec import AluOp as _AluOp

B_TOTAL, C_IN, H, W = 32, 64, 128, 128
PICK = 16
N_CORES = 8
B_PER = B_TOTAL // N_CORES
C_OUT = C_IN + 4 * PICK
F32 = mybir.dt.float32
BF16 = mybir.dt.bfloat16
NP_BF16 = ml_dtypes.bfloat16
CW = PICK * W  # 2048, free size of one direction's tile


# --- SEGMAX_SCAN: custom DVE op — segmented cummax along the free dim. ---
#
# Spec body is a plain max-scan; the subdim step state is hand-patched to
# reset the scan latch at each page boundary: on SUB_DIM_DONE the scan
# stage computes max(-FLT_MAX, x) = x for the first element of the new
# page, instead of max(state, x).  With in0/out as [P, S, N] APs
# (subdim=True, no AP coalescing), this is np.maximum.accumulate per
# page at 1 elem/cycle/partition.

def _segmax_ref(in0, in1, s0, s1, imm2):
    return np.maximum.accumulate(in0.astype(np.float32), axis=-1)


class _SegmaxDveOp(_do.DveOp):
    def compile(self, ver):
        key = (self.name, ver)
        if (r := _do._COMPILE_CACHE.get(key)) is not None:
            return r
        orig = _ds._scan_overrides

        def patched(scans, node_stage):
            seed, step = orig(scans, node_stage)
            for sc in scans:
                step[node_stage[sc]] = _ds._Stage(
                    _ds.AluOp.MAX, _ds.MaxNeg, _ds.Src0
                )
            return seed, step

        _ds._scan_overrides = patched
        try:
            uops = _ds.lower(self.spec, ver=ver)
        finally:
            _ds._scan_overrides = orig
        result = _do.DveOpSpec(
            name=self.name,
            opcode=_do.get_dve_sub_opcode(self.name),
            uops=uops,
            rd1_en=_do.has_src1(self.spec),
        )
        _do._COMPILE_CACHE[key] = result
        return result


def _get_segmax_op():
    for op in _do.OPS:
        if op.name == "SEGMAX_SCAN":
            return op
    op = _SegmaxDveOp(
        "SEGMAX_SCAN",
        _Spec(body=_scan(_AluOp.MAX, _Src0), reference=_segmax_ref),
        subdim=True,
        uops_sha={},
    )
    _do.OPS.append(op)
    _do._SUB_OPCODE_FOR_NAME[op.name] = _do._CUSTOM_DVE_ROW_BASE + _do.OPS.index(op)
    _do.CUSTOM_DVE_SPECS[op.name] = op.spec
    return op


def segmax_scan(nc, out3, in3):
    """out3/in3: [P, S, N] APs — cummax along N, reset per S page."""
    return nc.vector._custom_dve(_get_segmax_op(), out=out3, in0=in3)


# --- kernel body ---

def _emit(
    ctx: ExitStack,
    tc: TileContext,
    xh: bass.AP,
    xv: bass.AP,
    out_rl: bass.AP,
    out_du: bass.AP,
    reps: int = 1,
) -> None:
    nc = tc.nc

    in_pool = ctx.enter_context(tc.tile_pool(name="tin", bufs=3))
    out_pool = ctx.enter_context(tc.tile_pool(name="tout", bufs=3))

    def seg3(ap):
        return ap.rearrange("p (s n) -> p s n", n=W)

    HF = CW // 2  # 1024: an 8-channel half

    for _rep in range(reps):
        for b in range(B_PER):
            first = _rep == 0 and b == 0
            last = _rep == reps - 1 and b == B_PER - 1

            th = in_pool.tile([128, CW], BF16, tag="th")
            tv = in_pool.tile([128, CW], BF16, tag="tv")
            if first:
                # Split the first load so the first scan (channels 0-7)
                # starts after 256 KB instead of 512 KB.  The two halves
                # go out on different trigger engines so their queue
                # entries enqueue back-to-back, and the tv copy (not
                # needed until the 3rd scan) trails both.
                nc.sync.dma_start(out=th[:, 0:HF], in_=xh[b, :, 0:HF])
                nc.scalar.dma_start(out=th[:, HF:CW], in_=xh[b, :, HF:CW])
                nc.sync.dma_start(out=tv[:], in_=xv[b])
            else:
                nc.sync.dma_start(out=th[:], in_=xh[b])
                nc.scalar.dma_start(out=tv[:], in_=xv[b])

            orl = out_pool.tile([128, 2 * CW], BF16, tag="orl")
            odu = out_pool.tile([128, 2 * CW], BF16, tag="odu")
            # right: forward segmented cummax over [h, (c w)].
            if first:
                segmax_scan(nc, seg3(orl[:, 0:HF]), seg3(th[:, 0:HF]))
                segmax_scan(nc, seg3(orl[:, HF:CW]), seg3(th[:, HF:CW]))
            else:
                segmax_scan(nc, seg3(orl[:, 0:CW]), seg3(th[:]))
            nc.sync.dma_start(out=out_rl[b, :, 0:CW], in_=orl[:, 0:CW])
            # left: the same scan through fully-reversed in/out APs; the
            # two reversals cancel, so the region lands in natural (c, w).
            segmax_scan(
                nc,
                seg3(orl[:, CW : 2 * CW])[:, ::-1, ::-1],
                seg3(th[:])[:, ::-1, ::-1],
            )
            nc.scalar.dma_start(
                out=out_rl[b, :, CW : 2 * CW], in_=orl[:, CW : 2 * CW]
            )
            # down/up: identical over the host-transposed [w, (c h)] copy.
            segmax_scan(nc, seg3(odu[:, 0:CW]), seg3(tv[:]))
            nc.sync.dma_start(out=out_du[b, :, 0:CW], in_=odu[:, 0:CW])
            uo = seg3(odu[:, CW : 2 * CW])[:, ::-1, ::-1]
            ui = seg3(tv[:])[:, ::-1, ::-1]
            if last:
                # Split the final scan+store so the tail drains 256 KB.
                segmax_scan(nc, uo[:, 0:8], ui[:, 0:8])
                nc.scalar.dma_start(
                    out=out_du[b, :, CW + HF : 2 * CW],
                    in_=odu[:, CW + HF : 2 * CW],
                )
                segmax_scan(nc, uo[:, 8:16], ui[:, 8:16])
                nc.scalar.dma_start(
                    out=out_du[b, :, CW : CW + HF], in_=odu[:, CW : CW + HF]
                )
            else:
                segmax_scan(nc, uo, ui)
                nc.scalar.dma_start(
                    out=out_du[b, :, CW : 2 * CW], in_=odu[:, CW : 2 * CW]
                )


def build_nc(reps: int = 1) -> bass.Bass:
    # Bacc (not raw Bass): its compile() legalizes sync waits for TRN2.
    nc = bacc.Bacc("TRN2", target_bir_lowering=False, debug=False)
    # xh: picked channels as [b, h, (c w)]; xv: as [b, w, (c h)] (bf16).
    xh = nc.declare_dram_parameter("xh", [B_PER, H, CW], BF16, isOutput=False)
    xv = nc.declare_dram_parameter("xv", [B_PER, W, CW], BF16, isOutput=False)
    # out_rl: [b, h, (right|left, c, w)]; out_du: [b, w, (down|up, c, h)].
    out_rl = nc.declare_dram_parameter(
        "out_rl", [B_PER, H, 2 * CW], BF16, isOutput=True
    )
    out_du = nc.declare_dram_parameter(
        "out_du", [B_PER, W, 2 * CW], BF16, isOutput=True
    )
    with TileContext(nc) as tc:
        with ExitStack() as ctx:
            _emit(ctx, tc, xh, xv, out_rl, out_du, reps=reps)
    nc.compile()
    return nc


def make_in_maps(x: np.ndarray) -> list[dict[str, np.ndarray]]:
    p = x[:, :PICK]
    xh = np.ascontiguousarray(p.transpose(0, 2, 1, 3)).astype(NP_BF16)
    xv = np.ascontiguousarray(p.transpose(0, 3, 1, 2)).astype(NP_BF16)
    xh = xh.reshape(B_TOTAL, H, CW)
    xv = xv.reshape(B_TOTAL, W, CW)
    return [
        {
            "xh": xh[k * B_PER : (k + 1) * B_PER],
            "xv": xv[k * B_PER : (k + 1) * B_PER],
        }
        for k in range(N_CORES)
    ]


def kernel(x: np.ndarray, **_unused) -> np.ndarray:
    assert x.shape == (B_TOTAL, C_IN, H, W), x.shape
    nc = build_nc()
    res = run_bass_kernel_spmd(nc, make_in_maps(x), list(range(N_CORES)))

    out = np.empty((B_TOTAL, C_OUT, H, W), np.float32)
    out[:, :C_IN] = x
    rl = np.concatenate([r["out_rl"] for r in res.results], axis=0)
    du = np.concatenate([r["out_du"] for r in res.results], axis=0)
    rl = rl.reshape(B_TOTAL, H, 2, PICK, W).astype(np.float32)
    du = du.reshape(B_TOTAL, W, 2, PICK, H).astype(np.float32)
    out[:, C_IN : C_IN + PICK] = du[:, :, 0].transpose(0, 2, 3, 1)  # down
    out[:, C_IN + PICK : C_IN + 2 * PICK] = du[:, :, 1].transpose(0, 2, 3, 1)  # up
    out[:, C_IN + 2 * PICK : C_IN + 3 * PICK] = rl[:, :, 0].transpose(0, 2, 1, 3)
    out[:, C_IN + 3 * PICK :] = rl[:, :, 1].transpose(0, 2, 1, 3)  # left
    return out
